# revision 3
# baseline (speedup 1.0000x reference)
"""Gated GQA self-attention with KV cache, tensor-parallel over heads on 8
Trainium2 NeuronCores.

Reference computation (fp32):
    q = rms_norm((x @ w_q.T).reshape(B,L,H,HD))      # per-head rms over HD
    k = rms_norm((x @ w_k.T).reshape(B,L,HKV,HD))
    v = (x @ w_v.T).reshape(B,L,HKV,HD)
    k_t/v_t = concat(cache, new) over seq -> [B,HKV,S,HD]
    o = softmax(q @ k_t.T / sqrt(HD)) @ v_t          # full (non-causal)
    o *= sigmoid(x[..., :16] @ w_gate.T)             # per-head gate
    y = o.reshape(B,L,D) @ w_out.T

Sharding: core c owns q heads {2c, 2c+1} and kv group g=c//2 (GQA groups
stay intact).  Each core computes its heads' attention plus the partial
out-projection y_c = o_c @ w_out[:, cols_c].T; the host sums the 8
partials (replaces the all-reduce).

Device-side layout: everything is computed feature-on-partition
("transposed"), so scores come out [s, l] and the P matrix never needs a
transpose for the p@v matmul.  The host pre-transposes x and the weights
and pre-casts them to bf16, so all matmuls run bf16 (1 cycle/row, FWL
weight loads, half the HBM traffic of fp32) with fp32 PSUM accumulation.
Statistics (rms scales, softmax denominators, gates) stay fp32.

Softmax runs without max-subtraction (scores here are ~N(0,1); exp
cannot overflow).  Both rms-norm scales (q and k) are folded into the
q/k activations as rank-1 PE broadcasts before attention, so every exp
is a plain scale-free activation over a PAIR of score chunks (N=1024
amortizes the ACT engine's 352-cycle fixed overhead).  The softmax
denominator is a ones-matmul partition sum; gate/denominator factors are
folded into the attention-output evacuation as one more rank-1
broadcast, which lets the out-projection accumulate both heads into one
PSUM bank and stream straight out as bf16.  1-partition rows are
reshaped to [128, n] via small internal-DRAM bounce DMAs so reciprocals
never run on a single DVE lane.
"""

from contextlib import ExitStack

import numpy as np
import ml_dtypes

import concourse.bass as bass
import concourse.tile as tile
from concourse import bacc, mybir
from concourse.bass_utils import run_bass_kernel_spmd

BF16 = mybir.dt.bfloat16
F32R = mybir.dt.float32r
F32 = mybir.dt.float32
AF = mybir.ActivationFunctionType
NPBF16 = ml_dtypes.bfloat16

B, L, D = 2, 1024, 2048
H, HKV, HD = 16, 4, 128
CACHE = 1024
BL = B * L                  # 2048
S = CACHE + L               # 2048
NCORES = 8
QH = H // NCORES            # 2 q heads per core
JC = QH * HD                # 256 out-proj contraction cols per core
EPS = 1e-6

NLP = BL // 256             # 8 column chunks for the x stream
ND = D // 128               # 16 contraction chunks for the projections
NSC = CACHE // 128          # 8 cached s chunks per batch
NS = S // 128               # 16 s chunks per batch

_CACHED_NC = None


def _build_core_program():
    """One SPMD program; per-core differences are input data only."""
    nc = bacc.Bacc("TRN2", target_bir_lowering=False, debug=False)

    xt = nc.dram_tensor("xt", [D, BL], BF16, kind="ExternalInput").ap()
    wqkv = nc.dram_tensor("wqkv", [D, 4 * HD], BF16, kind="ExternalInput").ap()
    wo = nc.dram_tensor("wo", [JC, D], BF16, kind="ExternalInput").ap()
    wg = nc.dram_tensor("wg", [H, QH], BF16, kind="ExternalInput").ap()
    ckt = nc.dram_tensor("ckt", [B, HD, CACHE], BF16, kind="ExternalInput").ap()
    cv = nc.dram_tensor("cv", [B, CACHE, HD], BF16, kind="ExternalInput").ap()
    # [:, :128] identity for PE transposes, [:, 128] all-ones column
    consts_in = nc.dram_tensor("consts", [128, 129], BF16, kind="ExternalInput").ap()
    onesr_in = nc.dram_tensor("onesr", [1, 128], F32R, kind="ExternalInput").ap()
    y = nc.dram_tensor("y", [BL, D], BF16, kind="ExternalOutput").ap()

    # internal-DRAM bounce buffers for row<->column reshapes
    qscr = nc.dram_tensor("qscr", [QH, 16, 128], F32R).ap()
    kscr = nc.dram_tensor("kscr", [16, 128], F32R).ap()
    dscr = nc.dram_tensor("dscr", [8, 4, 128], F32R).ap()
    fscr = nc.dram_tensor("fscr", [8, 4, 128], F32R).ap()

    with tile.TileContext(nc) as tc, ExitStack() as ctx:
        singles = ctx.enter_context(tc.tile_pool(name="singles", bufs=1))
        xtp = ctx.enter_context(tc.tile_pool(name="xtp", bufs=2))
        work = ctx.enter_context(tc.tile_pool(name="work", bufs=4))
        cachep = ctx.enter_context(tc.tile_pool(name="cachep", bufs=1))
        frp = ctx.enter_context(tc.tile_pool(name="frp", bufs=2))
        colp = ctx.enter_context(tc.tile_pool(name="colp", bufs=2))

        # PSUM: psS 2x2-bank score pairs + psO 2x1 pv accumulators +
        # psD 1 softmax-denominator bank + misc 1 shared bank = 8 banks
        psS = ctx.enter_context(tc.tile_pool(name="psS", bufs=2, space="PSUM"))
        psO = ctx.enter_context(tc.tile_pool(name="psO", bufs=2, space="PSUM"))
        psD = ctx.enter_context(tc.tile_pool(name="psD", bufs=1, space="PSUM"))
        psM = ctx.enter_context(tc.tile_pool(name="psM", bufs=1, space="PSUM"))

        lowp = nc.allow_low_precision(reason="bf16 matmuls are intended")
        ctx.enter_context(lowp)

        consts = singles.tile([128, 129], BF16)
        nc.scalar.dma_start(out=consts, in_=consts_in)
        ident = consts[:, 0:128]
        ones_col = consts[:, 128:129]
        ones_row = singles.tile([1, 128], F32R)
        nc.scalar.dma_start(out=ones_row, in_=onesr_in)

        bias_q = singles.tile([1, 1], F32)
        nc.vector.memset(bias_q, HD * EPS)
        bias_k = singles.tile([1, 1], F32)
        nc.vector.memset(bias_k, EPS)

        wg_sb = singles.tile([H, QH], BF16)
        nc.scalar.dma_start(out=wg_sb, in_=wg)
        # split the weight load so the first projection matmuls start after
        # ~1/4 of it has landed; v and k column blocks first
        wqkv_sb = singles.tile([128, ND, 4 * HD], BF16)
        wqkv_r = wqkv.rearrange("(k p) j -> p k j", p=128)
        for jq in (3, 2, 0, 1):
            nc.scalar.dma_start(
                out=wqkv_sb[:, :, jq * 128 : jq * 128 + 128],
                in_=wqkv_r[:, :, jq * 128 : jq * 128 + 128],
            )
        wo_sb = singles.tile([128, QH, D], BF16)

        # persistent activations, feature-on-partition
        qkvt = singles.tile([128, 4, BL], BF16)       # jc: qh0, qh1, k, v
        otg = singles.tile([128, B, QH, 2, 512], BF16)  # gated attention out
        gcol = singles.tile([128, 16, QH], F32)       # gates, l-on-partition
        qs = [
            singles.tile([1, BL], F32R, tag=f"qs{i}", name=f"qs{i}")
            for i in range(QH)
        ]  # q rms scale rows (sqrt then reciprocal via bounce)
        ksr = singles.tile([1, BL], F32R)             # k sqrt / recip row
        frows = singles.tile([1, 8, 512], F32R)       # gate/den rows
        xg = singles.tile([H, BL], BF16)              # x[..., :16] for gates
        cache_tiles = {}
        vnews = {}

        def emit_prefetch():
            """Non-critical loads, queued after the first x tile so they
            don't delay the first projection matmul."""
            nc.scalar.dma_start(
                out=wo_sb, in_=wo.rearrange("(h p) m -> p h m", p=128)
            )
            nc.gpsimd.dma_start(out=xg, in_=xt[0:H, :])
            for b in range(B):
                ck_sb = cachep.tile(
                    [128, CACHE], BF16, tag=f"ck{b}", name=f"ck{b}"
                )
                nc.gpsimd.dma_start(out=ck_sb, in_=ckt[b])
                cv_sb = cachep.tile(
                    [128, NSC, HD], BF16, tag=f"cv{b}", name=f"cv{b}"
                )
                nc.gpsimd.dma_start(
                    out=cv_sb, in_=cv[b].rearrange("(i p) d -> p i d", p=128)
                )
                cache_tiles[b] = (ck_sb, cv_sb)

        # ---- phase 1: projections -------------------------------------
        def recip_row(scr_rows, col, row_src_sl, row_dst_sl):
            """row[1,1024] -> DRAM -> [128,8] col -> recip -> DRAM -> row."""
            nc.gpsimd.dma_start(out=scr_rows, in_=row_src_sl)
            nc.gpsimd.dma_start(out=col, in_=scr_rows.rearrange("c p -> p c"))
            nc.vector.reciprocal(col, col)
            nc.gpsimd.dma_start(out=scr_rows.rearrange("c p -> p c"), in_=col)
            nc.gpsimd.dma_start(
                out=row_dst_sl, in_=scr_rows.flatten().unsqueeze(0)
            )

        def finish_half(half):
            """Reciprocal of the rms rows via DRAM bounce (1-lane DVE rows
            are ~6us each), then fold both rms scales into q/k via rank-1
            PE broadcasts — per half so the second half overlaps remaining
            projection work."""
            rs = slice(half * 8, half * 8 + 8)
            row_sl = slice(half * 1024, half * 1024 + 1024)
            kc = colp.tile([128, 8], F32R, tag="kc", name=f"kc{half}")
            recip_row(kscr[rs], kc, ksr[:, row_sl], ksr[:, row_sl])
            for h in range(QH):
                qc = colp.tile([128, 8], F32R, tag="qc", name=f"qc{h}_{half}")
                recip_row(qscr[h, rs], qc, qs[h][:, row_sl], qs[h][:, row_sl])
            for jc in (2, 0, 1):  # k first: unblocks attention soonest
                row = ksr if jc == 2 else qs[jc]
                for lc in range(2):
                    sl = slice(half * 1024 + lc * 512, half * 1024 + lc * 512 + 512)
                    bc = psM.tile([128, 512], F32, tag="psM", name=f"bc{jc}")
                    nc.tensor.matmul(
                        bc, ones_row, row[:, sl], start=True, stop=True
                    )
                    nc.vector.tensor_mul(qkvt[:, jc, sl], qkvt[:, jc, sl], bc)

        xt_r = xt.rearrange("(k p) l -> p k l", p=128)

        def proj_chunk(lc):
            sl = slice(lc * 256, lc * 256 + 256)
            xtile = xtp.tile([128, ND, 256], BF16, tag="xt")
            for kq in range(4):
                nc.sync.dma_start(
                    out=xtile[:, kq * 4 : kq * 4 + 4, :],
                    in_=xt_r[:, kq * 4 : kq * 4 + 4, sl],
                )
            if lc == 0:
                emit_prefetch()
            for jc in (3, 2, 0, 1):  # v and k first: unblocks attention prep
                pp = psS.tile([128, 256], F32, tag="psS", name=f"pp{lc}_{jc}")
                for kk in range(ND):
                    nc.tensor.matmul(
                        pp,
                        wqkv_sb[:, kk, jc * 128 : jc * 128 + 128],
                        xtile[:, kk, :],
                        start=(kk == 0),
                        stop=(kk == ND - 1),
                    )
                nc.vector.tensor_copy(qkvt[:, jc, sl], pp)
                if jc < 3:  # q0, q1, k need sum over HD of the square
                    sq = work.tile([128, 256], BF16, tag="sq", name=f"sq{lc}_{jc}")
                    nc.vector.tensor_mul(sq, qkvt[:, jc, sl], qkvt[:, jc, sl])
                    ssq = psM.tile([1, 256], F32, tag="psM", name=f"ssq{lc}_{jc}")
                    nc.tensor.matmul(ssq, ones_col, sq, start=True, stop=True)
                    # q: sqrt(ssq + HD*eps) so the reciprocal also folds in
                    # the 1/sqrt(HD) score scale; k: sqrt(ssq/HD + eps).
                    row = qs[jc] if jc < QH else ksr
                    scale, bias = (1.0, bias_q) if jc < QH else (1.0 / HD, bias_k)
                    nc.scalar.activation(
                        row[:, sl], ssq, AF.Sqrt, bias=bias[:], scale=scale
                    )

        def emit_gates():
            # gates in column form: [l-part, chunk, head]
            gps = psM.tile([128, 16, QH], F32, tag="psM", name="gps")
            for c in range(16):
                nc.tensor.matmul(
                    gps[:, c, :],
                    xg[:, c * 128 : c * 128 + 128],
                    wg_sb,
                    start=True,
                    stop=True,
                )
            nc.scalar.activation(gcol, gps, AF.Sigmoid)

        def emit_vnew(b):
            boff = b * L
            vnew = cachep.tile([128, NSC, HD], BF16, tag=f"vnew{b}", name=f"vn{b}")
            for i in range(NSC):
                tp = psM.tile([128, 128], BF16, tag="psM", name=f"tp{b}_{i}")
                nc.tensor.transpose(
                    tp, qkvt[:, 3, boff + i * 128 : boff + i * 128 + 128], ident
                )
                nc.vector.tensor_copy(vnew[:, i, :], tp)
            vnews[b] = vnew

        # ---- phase 2: attention ---------------------------------------
        def attn_iter(b, h, lc2):
            it = (b * QH + h) * 2 + lc2
            boff = b * L
            off = boff + lc2 * 512
            ck_sb, cv_sb = cache_tiles[b]
            vnew = vnews[b]
            qsl = qkvt[:, h, off : off + 512]
            den = psD.tile([1, 512], F32, tag="psD", name=f"den{it}")
            ot = psO.tile([128, 512], F32, tag="psO", name=f"ot{it}")
            for p in range(NS // 2):
                ps2 = psS.tile(
                    [128, 2, 512], F32, tag="psS", name=f"ps{it}_{p}"
                )
                exs = work.tile(
                    [128, 2, 512], BF16, tag="ex", name=f"ex{it}_{p}"
                )
                for hf in range(2):
                    sc = 2 * p + hf
                    if sc < NSC:
                        kT = ck_sb[:, sc * 128 : sc * 128 + 128]
                    else:
                        j = boff + (sc - NSC) * 128
                        kT = qkvt[:, 2, j : j + 128]
                    nc.tensor.matmul(
                        ps2[:, hf, :], kT, qsl, start=True, stop=True
                    )
                # one exp over both chunks: N=1024 amortizes ACT overhead
                nc.scalar.activation(exs, ps2, AF.Exp)
                for hf in range(2):
                    sc = 2 * p + hf
                    vx = cv_sb[:, sc, :] if sc < NSC else vnew[:, sc - NSC, :]
                    nc.tensor.matmul(
                        den, ones_col, exs[:, hf, :],
                        start=(sc == 0), stop=(sc == NS - 1),
                    )
                    nc.tensor.matmul(
                        ot, vx, exs[:, hf, :],
                        start=(sc == 0), stop=(sc == NS - 1),
                    )
            # gate/denominator: bounce den to a column, recip, fold the
            # gate in, bounce back to a row, PE-broadcast, and apply while
            # evacuating ot -> otg (so phase 3 needs no per-head scaling)
            drow = frp.tile([1, 512], F32R, tag="drow", name=f"drow{it}")
            nc.vector.tensor_copy(drow, den)
            nc.gpsimd.dma_start(out=dscr[it], in_=drow)
            dcol = colp.tile([128, 4], F32R, tag="dcol", name=f"dcol{it}")
            nc.gpsimd.dma_start(out=dcol, in_=dscr[it].rearrange("c p -> p c"))
            nc.vector.reciprocal(dcol, dcol)
            nc.vector.tensor_mul(
                dcol, dcol, gcol[:, 8 * b + 4 * lc2 : 8 * b + 4 * lc2 + 4, h]
            )
            nc.gpsimd.dma_start(out=fscr[it].rearrange("c p -> p c"), in_=dcol)
            frow = frows[:, it, :]
            nc.gpsimd.dma_start(
                out=frow, in_=fscr[it].flatten().unsqueeze(0)
            )
            bcf = psM.tile([128, 512], F32, tag="psM", name=f"bcf{it}")
            nc.tensor.matmul(bcf, ones_row, frow, start=True, stop=True)
            bcs = work.tile([128, 512], F32, tag="bcs", name=f"bcs{it}")
            nc.scalar.copy(bcs, bcf)
            nc.vector.tensor_mul(otg[:, b, h, lc2, :], ot, bcs)

        # ---- phase 3: partial out-projection --------------------------
        def phase3_block(b, lc2, li):
            row0 = b * L + lc2 * 512 + li * 128
            for mc in range(4):
                yp = psS.tile(
                    [128, 512], F32, tag="psS", name=f"yp{b}_{lc2}_{li}_{mc}"
                )
                for h in range(QH):
                    nc.tensor.matmul(
                        yp,
                        otg[:, b, h, lc2, li * 128 : li * 128 + 128],
                        wo_sb[:, h, mc * 512 : mc * 512 + 512],
                        start=(h == 0),
                        stop=(h == QH - 1),
                    )
                ysb = work.tile(
                    [128, 512], BF16, tag="ysb", name=f"ysb{b}_{lc2}_{li}_{mc}"
                )
                nc.vector.tensor_copy(ysb, yp)
                eng = nc.sync if mc % 2 == 0 else nc.gpsimd
                eng.dma_start(
                    out=y[row0 : row0 + 128, mc * 512 : mc * 512 + 512],
                    in_=ysb,
                )

        # ---- emission order -------------------------------------------
        for lc in range(4):
            proj_chunk(lc)
        finish_half(0)
        for lc in range(4, NLP):
            proj_chunk(lc)
        finish_half(1)
        emit_gates()
        emit_vnew(0)
        for h in range(QH):
            for lc2 in range(2):
                attn_iter(0, h, lc2)
        emit_vnew(1)
        # interleave phase-3 b0 blocks with attention b1: the out-proj's
        # pure-PE work fills the engine while b1's exps run on ACT
        p3 = [(0, lc2, li) for lc2 in range(2) for li in range(4)]
        for i, (h, lc2) in enumerate([(0, 0), (0, 1), (1, 0), (1, 1)]):
            attn_iter(1, h, lc2)
            for blk in p3[i * 2 : i * 2 + 2]:
                phase3_block(*blk)
        for lc2 in range(2):
            for li in range(4):
                phase3_block(1, lc2, li)

    nc.compile()
    return nc


def _get_nc():
    global _CACHED_NC
    if _CACHED_NC is None:
        _CACHED_NC = _build_core_program()
    return _CACHED_NC


def make_in_maps(x, w_q, w_k, w_v, w_out, w_gate, cache_k, cache_v):
    xt = x.reshape(BL, D).T.astype(NPBF16)
    consts_np = np.concatenate(
        [np.eye(128, dtype=np.float32), np.ones((128, 1), np.float32)], axis=1
    ).astype(NPBF16)
    onesr_np = np.ones((1, 128), np.float32)
    in_maps = []
    for c in range(NCORES):
        g = c // 2
        wq_c = w_q[c * JC : (c + 1) * JC]                      # [256, D]
        wk_c = w_k[g * HD : (g + 1) * HD]                      # [128, D]
        wv_c = w_v[g * HD : (g + 1) * HD]
        wqkv_c = np.concatenate([wq_c, wk_c, wv_c], axis=0).T.astype(NPBF16)
        wo_c = w_out[:, c * JC : (c + 1) * JC].T.astype(NPBF16)  # [256, D]
        wg_c = w_gate[c * QH : (c + 1) * QH].T.astype(NPBF16)    # [16, 2]
        ckt_c = cache_k[:, g].transpose(0, 2, 1).astype(NPBF16)  # [B,HD,CACHE]
        cv_c = cache_v[:, g].astype(NPBF16)                      # [B,CACHE,HD]
        in_maps.append(
            {
                "xt": xt,
                "wqkv": wqkv_c,
                "wo": wo_c,
                "wg": wg_c,
                "ckt": ckt_c,
                "cv": cv_c,
                "consts": consts_np,
                "onesr": onesr_np,
            }
        )
    return in_maps


def kernel(x, w_q, w_k, w_v, w_out, w_gate, cache_k, cache_v, _run_kwargs=None):
    in_maps = make_in_maps(x, w_q, w_k, w_v, w_out, w_gate, cache_k, cache_v)
    nc = _get_nc()
    res = run_bass_kernel_spmd(
        nc, in_maps, core_ids=list(range(NCORES)), **(_run_kwargs or {})
    )
    acc = np.zeros((BL, D), dtype=np.float32)
    for c in range(NCORES):
        acc += res.results[c]["y"].astype(np.float32)
    out = acc.reshape(B, L, D)
    if _run_kwargs:
        kernel.last_results = res
    return out


# revision 4
# speedup vs baseline: 1.2373x; 1.2373x over previous
"""Gated GQA self-attention with KV cache, tensor-parallel over heads on 8
Trainium2 NeuronCores.

Reference computation (fp32):
    q = rms_norm((x @ w_q.T).reshape(B,L,H,HD))      # per-head rms over HD
    k = rms_norm((x @ w_k.T).reshape(B,L,HKV,HD))
    v = (x @ w_v.T).reshape(B,L,HKV,HD)
    k_t/v_t = concat(cache, new) over seq -> [B,HKV,S,HD]
    o = softmax(q @ k_t.T / sqrt(HD)) @ v_t          # full (non-causal)
    o *= sigmoid(x[..., :16] @ w_gate.T)             # per-head gate
    y = o.reshape(B,L,D) @ w_out.T

Sharding: core c owns q heads {2c, 2c+1} and kv group g=c//2 (GQA groups
stay intact).  Each core computes its heads' attention plus the partial
out-projection y_c = o_c @ w_out[:, cols_c].T; the host sums the 8
partials (replaces the all-reduce).

Device-side layout: everything is computed feature-on-partition
("transposed"), so scores come out [s, l] and the P matrix never needs a
transpose for the p@v matmul.  The host pre-transposes x and the weights
and pre-casts them to bf16, so all matmuls run bf16 (1 cycle/row, FWL
weight loads, half the HBM traffic of fp32) with fp32 PSUM accumulation.
Statistics (rms scales, softmax denominators, gates) stay fp32.

Softmax runs without max-subtraction (scores here are ~N(0,1); exp
cannot overflow).  Both rms-norm scales (q and k) are folded into the
q/k activations as rank-1 PE broadcasts before attention, so every exp
is a plain scale-free activation over a PAIR of score chunks (N=1024
amortizes the ACT engine's 352-cycle fixed overhead).  The softmax
denominator is a ones-matmul partition sum; gate/denominator factors are
folded into the attention-output evacuation as one more rank-1
broadcast, which lets the out-projection accumulate both heads into one
PSUM bank and stream straight out as bf16.  1-partition rows are
reshaped to [128, n] via small internal-DRAM bounce DMAs so reciprocals
never run on a single DVE lane.

The PE runs in program order, so anything that waits on a DMA bounce is
software-pipelined one step behind independent matmul work: the rank-1
broadcasts for the rms scales are woven between later projection chunks,
and each attention iteration's gate/den broadcast + evacuation is
emitted after the NEXT iteration's compute.
"""

from contextlib import ExitStack

import numpy as np
import ml_dtypes

import concourse.bass as bass
import concourse.tile as tile
from concourse import bacc, mybir
from concourse.bass_utils import run_bass_kernel_spmd

BF16 = mybir.dt.bfloat16
F32R = mybir.dt.float32r
F32 = mybir.dt.float32
AF = mybir.ActivationFunctionType
NPBF16 = ml_dtypes.bfloat16

B, L, D = 2, 1024, 2048
H, HKV, HD = 16, 4, 128
CACHE = 1024
BL = B * L                  # 2048
S = CACHE + L               # 2048
NCORES = 8
QH = H // NCORES            # 2 q heads per core
JC = QH * HD                # 256 out-proj contraction cols per core
EPS = 1e-6

NLP = BL // 256             # 8 column chunks for the x stream
ND = D // 128               # 16 contraction chunks for the projections
NSC = CACHE // 128          # 8 cached s chunks per batch
NS = S // 128               # 16 s chunks per batch

_CACHED_NC = None


def _build_core_program():
    """One SPMD program; per-core differences are input data only."""
    nc = bacc.Bacc("TRN2", target_bir_lowering=False, debug=False)

    xt = nc.dram_tensor("xt", [D, BL], BF16, kind="ExternalInput").ap()
    wqkv = nc.dram_tensor("wqkv", [D, 4 * HD], BF16, kind="ExternalInput").ap()
    wo = nc.dram_tensor("wo", [JC, D], BF16, kind="ExternalInput").ap()
    wg = nc.dram_tensor("wg", [H, QH], BF16, kind="ExternalInput").ap()
    ckt = nc.dram_tensor("ckt", [B, HD, CACHE], BF16, kind="ExternalInput").ap()
    cv = nc.dram_tensor("cv", [B, CACHE, HD], BF16, kind="ExternalInput").ap()
    # [:, :128] identity for PE transposes, [:, 128] all-ones column
    consts_in = nc.dram_tensor("consts", [128, 129], BF16, kind="ExternalInput").ap()
    onesr_in = nc.dram_tensor("onesr", [1, 128], F32R, kind="ExternalInput").ap()
    y = nc.dram_tensor("y", [BL, D], BF16, kind="ExternalOutput").ap()

    # internal-DRAM bounce buffers for row<->column reshapes
    qscr = nc.dram_tensor("qscr", [QH, 16, 128], F32R).ap()
    kscr = nc.dram_tensor("kscr", [16, 128], F32R).ap()
    dscr = nc.dram_tensor("dscr", [8, 4, 128], F32R).ap()
    fscr = nc.dram_tensor("fscr", [8, 4, 128], F32R).ap()

    with tile.TileContext(nc) as tc, ExitStack() as ctx:
        singles = ctx.enter_context(tc.tile_pool(name="singles", bufs=1))
        xtp = ctx.enter_context(tc.tile_pool(name="xtp", bufs=2))
        work = ctx.enter_context(tc.tile_pool(name="work", bufs=4))
        cachep = ctx.enter_context(tc.tile_pool(name="cachep", bufs=1))
        frp = ctx.enter_context(tc.tile_pool(name="frp", bufs=2))
        colp = ctx.enter_context(tc.tile_pool(name="colp", bufs=2))

        # PSUM: psS 2x2-bank score pairs + psO 2x1 pv accumulators +
        # psD 1 softmax-denominator bank + psM 1 shared bank = 8 banks
        psS = ctx.enter_context(tc.tile_pool(name="psS", bufs=2, space="PSUM"))
        psO = ctx.enter_context(tc.tile_pool(name="psO", bufs=2, space="PSUM"))
        psD = ctx.enter_context(tc.tile_pool(name="psD", bufs=1, space="PSUM"))
        psM = ctx.enter_context(tc.tile_pool(name="psM", bufs=1, space="PSUM"))

        lowp = nc.allow_low_precision(reason="bf16 matmuls are intended")
        ctx.enter_context(lowp)

        consts = singles.tile([128, 129], BF16)
        nc.scalar.dma_start(out=consts, in_=consts_in)
        ident = consts[:, 0:128]
        ones_col = consts[:, 128:129]
        ones_row = singles.tile([1, 128], F32R)
        nc.scalar.dma_start(out=ones_row, in_=onesr_in)

        bias_q = singles.tile([1, 1], F32)
        nc.vector.memset(bias_q, HD * EPS)
        bias_k = singles.tile([1, 1], F32)
        nc.vector.memset(bias_k, EPS)

        wg_sb = singles.tile([H, QH], BF16)
        nc.scalar.dma_start(out=wg_sb, in_=wg)
        # split the weight load so the first projection matmuls start after
        # a fraction of it has landed; v and k column blocks first
        wqkv_sb = singles.tile([128, ND, 4 * HD], BF16)
        wqkv_r = wqkv.rearrange("(k p) j -> p k j", p=128)
        nc.scalar.dma_start(
            out=wqkv_sb[:, 0:4, 384:512], in_=wqkv_r[:, 0:4, 384:512]
        )
        nc.scalar.dma_start(
            out=wqkv_sb[:, 4:16, 384:512], in_=wqkv_r[:, 4:16, 384:512]
        )
        for jq in (2, 0, 1):
            nc.scalar.dma_start(
                out=wqkv_sb[:, :, jq * 128 : jq * 128 + 128],
                in_=wqkv_r[:, :, jq * 128 : jq * 128 + 128],
            )
        wo_sb = singles.tile([128, QH, D], BF16)

        # persistent activations, feature-on-partition
        qkvt = singles.tile([128, 4, BL], BF16)       # jc: qh0, qh1, k, v
        otg = singles.tile([128, B, QH, 2, 512], BF16)  # gated attention out
        gcol = singles.tile([128, 16, QH], F32)       # gates, l-on-partition
        qs = [
            singles.tile([1, BL], F32R, tag=f"qs{i}", name=f"qs{i}")
            for i in range(QH)
        ]  # q rms scale rows (sqrt then reciprocal via bounce)
        ksr = singles.tile([1, BL], F32R)             # k sqrt / recip row
        frows = singles.tile([1, 8, 512], F32R)       # gate/den rows
        xg = singles.tile([H, BL], BF16)              # x[..., :16] for gates
        cache_tiles = {}
        vnews = {}

        def emit_prefetch():
            """Non-critical loads, queued after the first x tile so they
            don't delay the first projection matmul."""
            nc.scalar.dma_start(
                out=wo_sb, in_=wo.rearrange("(h p) m -> p h m", p=128)
            )
            nc.gpsimd.dma_start(out=xg, in_=xt[0:H, :])
            for b in range(B):
                ck_sb = cachep.tile(
                    [128, CACHE], BF16, tag=f"ck{b}", name=f"ck{b}"
                )
                nc.gpsimd.dma_start(out=ck_sb, in_=ckt[b])
                cv_sb = cachep.tile(
                    [128, NSC, HD], BF16, tag=f"cv{b}", name=f"cv{b}"
                )
                nc.gpsimd.dma_start(
                    out=cv_sb, in_=cv[b].rearrange("(i p) d -> p i d", p=128)
                )
                cache_tiles[b] = (ck_sb, cv_sb)

        # ---- phase 1: projections -------------------------------------
        def recip_row(scr_rows, col, row_sl):
            """row[1,1024] -> DRAM -> [128,8] col -> recip -> DRAM -> row.
            DMA/DVE only — no PE work, so it can hide under anything."""
            nc.gpsimd.dma_start(out=scr_rows, in_=row_sl)
            nc.gpsimd.dma_start(out=col, in_=scr_rows.rearrange("c p -> p c"))
            nc.vector.reciprocal(col, col)
            nc.gpsimd.dma_start(out=scr_rows.rearrange("c p -> p c"), in_=col)
            nc.gpsimd.dma_start(out=row_sl, in_=scr_rows.flatten().unsqueeze(0))

        def emit_bounces(half):
            """Reciprocal of the rms rows via DRAM bounce (1-lane DVE rows
            are ~6us each)."""
            rs = slice(half * 8, half * 8 + 8)
            row_sl = slice(half * 1024, half * 1024 + 1024)
            kc = colp.tile([128, 8], F32R, tag="kc", name=f"kc{half}")
            recip_row(kscr[rs], kc, ksr[:, row_sl])
            for h in range(QH):
                qc = colp.tile([128, 8], F32R, tag="qc", name=f"qc{h}_{half}")
                recip_row(qscr[h, rs], qc, qs[h][:, row_sl])

        def bc_pair(half, jc, lc):
            """Fold one rms-scale row into q/k via a rank-1 PE broadcast."""
            row = ksr if jc == 2 else qs[jc]
            sl = slice(half * 1024 + lc * 512, half * 1024 + lc * 512 + 512)
            bc = psM.tile([128, 512], F32, tag="psM", name=f"bc{half}_{jc}_{lc}")
            nc.tensor.matmul(bc, ones_row, row[:, sl], start=True, stop=True)
            nc.vector.tensor_mul(qkvt[:, jc, sl], qkvt[:, jc, sl], bc)

        BC_PAIRS = [(jc, lc) for jc in (2, 0, 1) for lc in range(2)]

        xt_r = xt.rearrange("(k p) l -> p k l", p=128)

        def proj_chunk(lc):
            sl = slice(lc * 256, lc * 256 + 256)
            xtile = xtp.tile([128, ND, 256], BF16, tag="xt")
            for kq in range(4):
                nc.sync.dma_start(
                    out=xtile[:, kq * 4 : kq * 4 + 4, :],
                    in_=xt_r[:, kq * 4 : kq * 4 + 4, sl],
                )
            if lc == 0:
                emit_prefetch()
            for jc in (3, 2, 0, 1):  # v and k first: unblocks attention prep
                pp = psS.tile([128, 256], F32, tag="psS", name=f"pp{lc}_{jc}")
                for kk in range(ND):
                    nc.tensor.matmul(
                        pp,
                        wqkv_sb[:, kk, jc * 128 : jc * 128 + 128],
                        xtile[:, kk, :],
                        start=(kk == 0),
                        stop=(kk == ND - 1),
                    )
                # alternate evacuation between ACT and DVE to split the load
                if jc % 2 == 0:
                    nc.scalar.copy(qkvt[:, jc, sl], pp)
                else:
                    nc.vector.tensor_copy(qkvt[:, jc, sl], pp)
                if jc < 3:  # q0, q1, k need sum over HD of the square
                    sq = work.tile([128, 256], BF16, tag="sq", name=f"sq{lc}_{jc}")
                    nc.vector.tensor_mul(sq, qkvt[:, jc, sl], qkvt[:, jc, sl])
                    ssq = psM.tile([1, 256], F32, tag="psM", name=f"ssq{lc}_{jc}")
                    nc.tensor.matmul(ssq, ones_col, sq, start=True, stop=True)
                    # q: sqrt(ssq + HD*eps) so the reciprocal also folds in
                    # the 1/sqrt(HD) score scale; k: sqrt(ssq/HD + eps).
                    row = qs[jc] if jc < QH else ksr
                    scale, bias = (1.0, bias_q) if jc < QH else (1.0 / HD, bias_k)
                    nc.scalar.activation(
                        row[:, sl], ssq, AF.Sqrt, bias=bias[:], scale=scale
                    )

        def emit_gates():
            # gates in column form: [l-part, chunk, head]
            gps = psM.tile([128, 16, QH], F32, tag="psM", name="gps")
            for c in range(16):
                nc.tensor.matmul(
                    gps[:, c, :],
                    xg[:, c * 128 : c * 128 + 128],
                    wg_sb,
                    start=True,
                    stop=True,
                )
            nc.scalar.activation(gcol, gps, AF.Sigmoid)

        def emit_vnew(b):
            boff = b * L
            vnew = cachep.tile([128, NSC, HD], BF16, tag=f"vnew{b}", name=f"vn{b}")
            for i in range(NSC):
                tp = psM.tile([128, 128], BF16, tag="psM", name=f"tp{b}_{i}")
                nc.tensor.transpose(
                    tp, qkvt[:, 3, boff + i * 128 : boff + i * 128 + 128], ident
                )
                nc.vector.tensor_copy(vnew[:, i, :], tp)
            vnews[b] = vnew

        # ---- phase 2: attention ---------------------------------------
        def attn_compute(b, h, lc2):
            """Score/exp/den/pv stream plus the den->gate/den-row bounce.
            The PE-dependent epilogue is emitted separately (one iteration
            later) so the PE never waits on the bounce DMAs."""
            it = (b * QH + h) * 2 + lc2
            boff = b * L
            off = boff + lc2 * 512
            ck_sb, cv_sb = cache_tiles[b]
            vnew = vnews[b]
            qsl = qkvt[:, h, off : off + 512]
            den = psD.tile([1, 512], F32, tag="psD", name=f"den{it}")
            ot = psO.tile([128, 512], F32, tag="psO", name=f"ot{it}")
            for p in range(NS // 2):
                ps2 = psS.tile(
                    [128, 2, 512], F32, tag="psS", name=f"ps{it}_{p}"
                )
                exs = work.tile(
                    [128, 2, 512], BF16, tag="ex", name=f"ex{it}_{p}"
                )
                for hf in range(2):
                    sc = 2 * p + hf
                    if sc < NSC:
                        kT = ck_sb[:, sc * 128 : sc * 128 + 128]
                    else:
                        j = boff + (sc - NSC) * 128
                        kT = qkvt[:, 2, j : j + 128]
                    nc.tensor.matmul(
                        ps2[:, hf, :], kT, qsl, start=True, stop=True
                    )
                # one exp over both chunks: N=1024 amortizes ACT overhead
                nc.scalar.activation(exs, ps2, AF.Exp)
                for hf in range(2):
                    sc = 2 * p + hf
                    vx = cv_sb[:, sc, :] if sc < NSC else vnew[:, sc - NSC, :]
                    nc.tensor.matmul(
                        den, ones_col, exs[:, hf, :],
                        start=(sc == 0), stop=(sc == NS - 1),
                    )
                    nc.tensor.matmul(
                        ot, vx, exs[:, hf, :],
                        start=(sc == 0), stop=(sc == NS - 1),
                    )
            # bounce den to a column, recip, fold the gate in, bounce back
            # to a row (DMA/DVE only)
            drow = frp.tile([1, 512], F32R, tag="drow", name=f"drow{it}")
            nc.vector.tensor_copy(drow, den)
            nc.gpsimd.dma_start(out=dscr[it], in_=drow)
            dcol = colp.tile([128, 4], F32R, tag="dcol", name=f"dcol{it}")
            nc.gpsimd.dma_start(out=dcol, in_=dscr[it].rearrange("c p -> p c"))
            nc.vector.reciprocal(dcol, dcol)
            nc.vector.tensor_mul(
                dcol, dcol, gcol[:, 8 * b + 4 * lc2 : 8 * b + 4 * lc2 + 4, h]
            )
            nc.gpsimd.dma_start(out=fscr[it].rearrange("c p -> p c"), in_=dcol)
            nc.gpsimd.dma_start(
                out=frows[:, it, :], in_=fscr[it].flatten().unsqueeze(0)
            )
            return ot

        def attn_epilogue(b, h, lc2, ot):
            """PE-broadcast the gate/den row and apply it while evacuating
            ot -> otg (so phase 3 needs no per-head scaling)."""
            it = (b * QH + h) * 2 + lc2
            bcf = psM.tile([128, 512], F32, tag="psM", name=f"bcf{it}")
            nc.tensor.matmul(bcf, ones_row, frows[:, it, :], start=True, stop=True)
            bcs = work.tile([128, 512], F32, tag="bcs", name=f"bcs{it}")
            nc.scalar.copy(bcs, bcf)
            nc.vector.tensor_mul(otg[:, b, h, lc2, :], ot, bcs)

        # ---- phase 3: partial out-projection --------------------------
        def phase3_block(b, lc2, li):
            row0 = b * L + lc2 * 512 + li * 128
            for mc in range(4):
                yp = psS.tile(
                    [128, 512], F32, tag="psS", name=f"yp{b}_{lc2}_{li}_{mc}"
                )
                for h in range(QH):
                    nc.tensor.matmul(
                        yp,
                        otg[:, b, h, lc2, li * 128 : li * 128 + 128],
                        wo_sb[:, h, mc * 512 : mc * 512 + 512],
                        start=(h == 0),
                        stop=(h == QH - 1),
                    )
                ysb = work.tile(
                    [128, 512], BF16, tag="ysb", name=f"ysb{b}_{lc2}_{li}_{mc}"
                )
                # alternate evacuation engine and DMA queue per column chunk
                if mc % 2 == 0:
                    nc.scalar.copy(ysb, yp)
                else:
                    nc.vector.tensor_copy(ysb, yp)
                eng = nc.sync if mc % 2 == 0 else nc.gpsimd
                eng.dma_start(
                    out=y[row0 : row0 + 128, mc * 512 : mc * 512 + 512],
                    in_=ysb,
                )

        # ---- emission order -------------------------------------------
        for lc in range(4):
            proj_chunk(lc)
        emit_bounces(0)          # half-0 recip bounces hide under lc4-7
        proj_chunk(4)
        # weave the half-0 rank-1 broadcasts between projection chunks so
        # their DVE follow-ons never leave the PE idle
        for lc, i in ((5, 0), (6, 2), (7, 4)):
            proj_chunk(lc)
            for jc, l2 in BC_PAIRS[i : i + 2]:
                bc_pair(0, jc, l2)
        emit_bounces(1)
        emit_gates()
        emit_vnew(0)
        # attention b0; half-1 broadcasts woven between iterations
        its0 = [(0, h, lc2) for h in range(QH) for lc2 in range(2)]
        ots = {}
        prev = None
        for i, (b, h, lc2) in enumerate(its0):
            ots[(b, h, lc2)] = attn_compute(b, h, lc2)
            if prev is not None:
                attn_epilogue(*prev, ots[prev])
            for jc, l2 in BC_PAIRS[2 * i : 2 * i + 2]:
                bc_pair(1, jc, l2)
            prev = (b, h, lc2)
        emit_vnew(1)
        # attention b1 with phase-3 b0 blocks woven in: the out-proj's
        # pure-PE work fills the engine while b1's exps run on ACT
        p3_b0 = [(0, lc2, li) for lc2 in range(2) for li in range(4)]
        for i, (h, lc2) in enumerate([(0, 0), (0, 1), (1, 0), (1, 1)]):
            ots[(1, h, lc2)] = attn_compute(1, h, lc2)
            attn_epilogue(*prev, ots[prev])
            prev = (1, h, lc2)
            for blk in p3_b0[i * 2 : i * 2 + 2]:
                phase3_block(*blk)
        # drain: b1 lc2=0 out-proj while the last epilogue's bounce lands
        for li in range(4):
            phase3_block(1, 0, li)
        attn_epilogue(*prev, ots[prev])
        for li in range(4):
            phase3_block(1, 1, li)

    nc.compile()
    return nc


def _get_nc():
    global _CACHED_NC
    if _CACHED_NC is None:
        _CACHED_NC = _build_core_program()
    return _CACHED_NC


def make_in_maps(x, w_q, w_k, w_v, w_out, w_gate, cache_k, cache_v):
    xt = x.reshape(BL, D).T.astype(NPBF16)
    consts_np = np.concatenate(
        [np.eye(128, dtype=np.float32), np.ones((128, 1), np.float32)], axis=1
    ).astype(NPBF16)
    onesr_np = np.ones((1, 128), np.float32)
    in_maps = []
    for c in range(NCORES):
        g = c // 2
        wq_c = w_q[c * JC : (c + 1) * JC]                      # [256, D]
        wk_c = w_k[g * HD : (g + 1) * HD]                      # [128, D]
        wv_c = w_v[g * HD : (g + 1) * HD]
        wqkv_c = np.concatenate([wq_c, wk_c, wv_c], axis=0).T.astype(NPBF16)
        wo_c = w_out[:, c * JC : (c + 1) * JC].T.astype(NPBF16)  # [256, D]
        wg_c = w_gate[c * QH : (c + 1) * QH].T.astype(NPBF16)    # [16, 2]
        ckt_c = cache_k[:, g].transpose(0, 2, 1).astype(NPBF16)  # [B,HD,CACHE]
        cv_c = cache_v[:, g].astype(NPBF16)                      # [B,CACHE,HD]
        in_maps.append(
            {
                "xt": xt,
                "wqkv": wqkv_c,
                "wo": wo_c,
                "wg": wg_c,
                "ckt": ckt_c,
                "cv": cv_c,
                "consts": consts_np,
                "onesr": onesr_np,
            }
        )
    return in_maps


def kernel(x, w_q, w_k, w_v, w_out, w_gate, cache_k, cache_v, _run_kwargs=None):
    in_maps = make_in_maps(x, w_q, w_k, w_v, w_out, w_gate, cache_k, cache_v)
    nc = _get_nc()
    res = run_bass_kernel_spmd(
        nc, in_maps, core_ids=list(range(NCORES)), **(_run_kwargs or {})
    )
    acc = np.zeros((BL, D), dtype=np.float32)
    for c in range(NCORES):
        acc += res.results[c]["y"].astype(np.float32)
    out = acc.reshape(B, L, D)
    if _run_kwargs:
        kernel.last_results = res
    return out


# revision 32
# speedup vs baseline: 1.8605x; 1.5037x over previous
"""Gated GQA self-attention with KV cache, tensor-parallel over heads on 8
Trainium2 NeuronCores.

Reference computation (fp32):
    q = rms_norm((x @ w_q.T).reshape(B,L,H,HD))      # per-head rms over HD
    k = rms_norm((x @ w_k.T).reshape(B,L,HKV,HD))
    v = (x @ w_v.T).reshape(B,L,HKV,HD)
    k_t/v_t = concat(cache, new) over seq -> [B,HKV,S,HD]
    o = softmax(q @ k_t.T / sqrt(HD)) @ v_t          # full (non-causal)
    o *= sigmoid(x[..., :16] @ w_gate.T)             # per-head gate
    y = o.reshape(B,L,D) @ w_out.T

Sharding: core c owns q heads {2c, 2c+1} and kv group g=c//2 (GQA groups
stay intact).  Each core computes its heads' attention plus the partial
out-projection y_c = o_c @ w_out[:, cols_c].T; the host sums the 8
partials (replaces the all-reduce).

Device-side layout: everything is computed feature-on-partition
("transposed"), so scores come out [s, l] and the P matrix never needs a
transpose for the p@v matmul.  The host pre-transposes x and the weights
and pre-casts them to bf16, so all matmuls run bf16 (1 cycle/row, FWL
weight loads, half the HBM traffic of fp32) with fp32 PSUM accumulation.
Statistics (rms scales, softmax denominators, gates) stay fp32.

Softmax runs without max-subtraction (scores here are ~N(0,1); exp
cannot overflow).  Both rms-norm scales (q and k) are folded into the
q/k activations as rank-1 PE broadcasts before attention, so every exp
is a plain scale-free activation over a PAIR of score chunks (N=1024
amortizes the ACT engine's 352-cycle fixed overhead).  The softmax
denominator is a ones-matmul partition sum; gate/denominator factors are
folded into the attention-output evacuation as one more rank-1
broadcast, which lets the out-projection accumulate both heads into one
PSUM bank and stream straight out as bf16.  All row reciprocals use the
single-pass DVE `reciprocal_approx_fast` (~51 ULP, plenty under bf16
noise) so nothing ever bounces through DRAM.

The PE runs in program order, so anything with an off-PE dependency is
software-pipelined behind independent matmul work: the rank-1 broadcasts
for the rms scales, the v transposes, and the gate rows are woven
between projection chunks / attention iterations, and each attention
iteration's gate/den broadcast + evacuation is emitted after the NEXT
iteration's compute.
"""

from contextlib import ExitStack

import numpy as np
import ml_dtypes

import concourse.bass as bass
import concourse.tile as tile
from concourse import bacc, mybir
from concourse.bass_utils import run_bass_kernel_spmd

BF16 = mybir.dt.bfloat16
F32R = mybir.dt.float32r
F32 = mybir.dt.float32
AF = mybir.ActivationFunctionType
NPBF16 = ml_dtypes.bfloat16

B, L, D = 2, 1024, 2048
H, HKV, HD = 16, 4, 128
CACHE = 1024
BL = B * L                  # 2048
S = CACHE + L               # 2048
NCORES = 8
QH = H // NCORES            # 2 q heads per core
JC = QH * HD                # 256 out-proj contraction cols per core
EPS = 1e-6

NLP = BL // 256             # 8 column chunks for the x stream
ND = D // 128               # 16 contraction chunks for the projections
NSC = CACHE // 128          # 8 cached s chunks per batch
NS = S // 128               # 16 s chunks per batch

_CACHED_NC = None


def _build_core_program():
    """One SPMD program; per-core differences are input data only."""
    nc = bacc.Bacc("TRN2", target_bir_lowering=False, debug=False)

    xt = nc.dram_tensor("xt", [D, BL], BF16, kind="ExternalInput").ap()
    # host pre-groups wqkv by output block: [4(jq), 128(p), ND, HD] so each
    # jq block is one contiguous DMA
    wqkv = nc.dram_tensor("wqkv", [4, 128, ND, HD], BF16, kind="ExternalInput").ap()
    wo = nc.dram_tensor("wo", [JC, D], BF16, kind="ExternalInput").ap()
    wg = nc.dram_tensor("wg", [H, QH], BF16, kind="ExternalInput").ap()
    ckt = nc.dram_tensor("ckt", [B, HD, CACHE], BF16, kind="ExternalInput").ap()
    cv = nc.dram_tensor("cv", [B, CACHE, HD], BF16, kind="ExternalInput").ap()
    # [:, :128] identity for PE transposes, [:, 128] all-ones column
    consts_in = nc.dram_tensor("consts", [128, 129], BF16, kind="ExternalInput").ap()
    onesr_in = nc.dram_tensor("onesr", [1, 128], F32R, kind="ExternalInput").ap()
    y = nc.dram_tensor("y", [BL, D], BF16, kind="ExternalOutput").ap()

    with tile.TileContext(nc) as tc, ExitStack() as ctx:
        singles = ctx.enter_context(tc.tile_pool(name="singles", bufs=1))
        xtp = ctx.enter_context(tc.tile_pool(name="xtp", bufs=7))
        work = ctx.enter_context(tc.tile_pool(name="work", bufs=4))
        cachep = ctx.enter_context(tc.tile_pool(name="cachep", bufs=1))
        frp = ctx.enter_context(tc.tile_pool(name="frp", bufs=2))

        # PSUM: psS 2x2-bank score pairs + psO 2x1 pv accumulators +
        # psD 1 softmax-denominator bank + psM 1 shared bank = 8 banks
        psS = ctx.enter_context(tc.tile_pool(name="psS", bufs=2, space="PSUM"))
        psO = ctx.enter_context(tc.tile_pool(name="psO", bufs=2, space="PSUM"))
        psD = ctx.enter_context(tc.tile_pool(name="psD", bufs=1, space="PSUM"))
        psM = ctx.enter_context(tc.tile_pool(name="psM", bufs=1, space="PSUM"))

        lowp = nc.allow_low_precision(reason="bf16 matmuls are intended")
        ctx.enter_context(lowp)

        # weight load first in queue order: the very first projection
        # matmul gates on wqkv[3, :, 0:4]; everything else can land later
        wqkv_sb = singles.tile([128, 4, ND, HD], BF16)
        nc.scalar.dma_start(out=wqkv_sb[:, 3, 0:1], in_=wqkv[3, :, 0:1])
        nc.scalar.dma_start(out=wqkv_sb[:, 3, 1:4], in_=wqkv[3, :, 1:4])
        nc.scalar.dma_start(out=wqkv_sb[:, 3, 4:16], in_=wqkv[3, :, 4:16])
        for jq in (2, 0, 1):
            nc.scalar.dma_start(out=wqkv_sb[:, jq], in_=wqkv[jq])

        consts = singles.tile([128, 129], BF16)
        nc.gpsimd.dma_start(out=consts, in_=consts_in)
        ident = consts[:, 0:128]
        ones_col = consts[:, 128:129]
        ones_row = singles.tile([1, 128], F32R)
        nc.gpsimd.dma_start(out=ones_row, in_=onesr_in)

        bias_q = singles.tile([1, 1], F32)
        nc.vector.memset(bias_q, HD * EPS)
        bias_k = singles.tile([1, 1], F32)
        nc.vector.memset(bias_k, EPS)

        wg_sb = singles.tile([H, QH], BF16)
        nc.gpsimd.dma_start(out=wg_sb, in_=wg)
        wo_sb = singles.tile([128, QH, D], BF16)

        # persistent activations, feature-on-partition
        qkvt = singles.tile([128, 4, BL], BF16)       # jc: qh0, qh1, k, v
        otg = singles.tile([128, B, QH, 2, 512], BF16)  # gated attention out
        grows = [
            singles.tile([1, BL], BF16, tag=f"grow{i}", name=f"grow{i}")
            for i in range(QH)
        ]  # per-head sigmoid gate rows (partition 0)
        qs = [
            singles.tile([1, BL], F32, tag=f"qs{i}", name=f"qs{i}")
            for i in range(QH)
        ]  # q rms scale rows: sqrt via ACT, then in-place approx recip
        ksr = singles.tile([1, BL], F32)              # k rms scale row
        qs_bf = [
            singles.tile([1, BL], F32R, tag=f"qsb{i}", name=f"qsb{i}")
            for i in range(QH)
        ]
        ksr_bf = singles.tile([1, BL], F32R)          # f32r rows for PE bcast

        xg = singles.tile([H, BL], BF16)              # x[..., :16] for gates
        cache_tiles = {}
        vnews = {}

        def emit_prefetch():
            """Non-critical loads, queued after the first x tile so they
            don't delay the first projection matmul."""
            nc.scalar.dma_start(
                out=wo_sb, in_=wo.rearrange("(h p) m -> p h m", p=128)
            )
            nc.gpsimd.dma_start(out=xg, in_=xt[0:H, :])
            for b in range(B):
                ck_sb = cachep.tile(
                    [128, CACHE], BF16, tag=f"ck{b}", name=f"ck{b}"
                )
                nc.gpsimd.dma_start(out=ck_sb, in_=ckt[b])
                cv_sb = cachep.tile(
                    [128, NSC, HD], BF16, tag=f"cv{b}", name=f"cv{b}"
                )
                nc.gpsimd.dma_start(
                    out=cv_sb, in_=cv[b].rearrange("(i p) d -> p i d", p=128)
                )
                cache_tiles[b] = (ck_sb, cv_sb)

        # ---- phase 1: projections -------------------------------------
        def recip_rows(half):
            """In-place 1/row for the three rms sqrt rows of one half —
            single-pass DVE approx, no PE, no DMA."""
            row_sl = slice(half * 1024, half * 1024 + 1024)
            for row, row_bf in ((ksr, ksr_bf), (qs[0], qs_bf[0]), (qs[1], qs_bf[1])):
                nc.vector.reciprocal_approx_fast(row[:, row_sl], row[:, row_sl])
                nc.vector.tensor_copy(row_bf[:, row_sl], row[:, row_sl])

        def bc_pair(half, jc, lc):
            """Fold one rms-scale row into q/k via a rank-1 PE broadcast."""
            row = ksr_bf if jc == 2 else qs_bf[jc]
            sl = slice(half * 1024 + lc * 512, half * 1024 + lc * 512 + 512)
            bc = psM.tile([128, 512], F32, tag="psM", name=f"bc{half}_{jc}_{lc}")
            nc.tensor.matmul(bc, ones_row, row[:, sl], start=True, stop=True)
            nc.vector.tensor_mul(qkvt[:, jc, sl], qkvt[:, jc, sl], bc)

        BC_PAIRS = [(jc, lc) for jc in (2, 0, 1) for lc in range(2)]

        xt_r = xt.rearrange("(k p) l -> p k l", p=128)
        xtiles = {}

        def proj_chunk(lc, jcs):
            sl = slice(lc * 256, lc * 256 + 256)
            if lc not in xtiles:
                xtile = xtp.tile([128, ND, 256], BF16, tag="xt", name=f"xt{lc}")
                for kq in range(4):
                    nc.sync.dma_start(
                        out=xtile[:, kq * 4 : kq * 4 + 4, :],
                        in_=xt_r[:, kq * 4 : kq * 4 + 4, sl],
                    )
                xtiles[lc] = xtile
                if lc == 0:
                    emit_prefetch()
            xtile = xtiles[lc]
            for jc in jcs:
                pp = psS.tile([128, 256], F32, tag="psS", name=f"pp{lc}_{jc}")
                for kk in range(ND):
                    nc.tensor.matmul(
                        pp,
                        wqkv_sb[:, jc, kk],
                        xtile[:, kk, :],
                        start=(kk == 0),
                        stop=(kk == ND - 1),
                    )
                nc.vector.tensor_copy(qkvt[:, jc, sl], pp)
                if jc < 3:  # q0, q1, k need sum over HD of the square
                    sq = work.tile([128, 256], BF16, tag="sq", name=f"sq{lc}_{jc}", bufs=2)
                    nc.vector.tensor_mul(sq, qkvt[:, jc, sl], qkvt[:, jc, sl])
                    ssq = psD.tile([1, 256], F32, tag="psD", name=f"ssq{lc}_{jc}")
                    nc.tensor.matmul(ssq, ones_col, sq, start=True, stop=True)
                    # q: sqrt(ssq + HD*eps) so the reciprocal also folds in
                    # the 1/sqrt(HD) score scale; k: sqrt(ssq/HD + eps).
                    row = qs[jc] if jc < QH else ksr
                    scale, bias = (1.0, bias_q) if jc < QH else (1.0 / HD, bias_k)
                    nc.scalar.activation(
                        row[:, sl], ssq, AF.Sqrt, bias=bias[:], scale=scale
                    )

        GATE_PARTS = [(c, h) for c in range(4) for h in range(QH)]

        def gates_part(i0, i1):
            # per-head gate rows [1, l] so they multiply straight onto den
            # rows without leaving partition 0.  sigmoid = 1/(1+exp(-x))
            # via the exp table set (sigmoid's own set would thrash the
            # ACT table RAM against the attention exps)
            for c, h in GATE_PARTS[i0:i1]:
                gsl = slice(c * 512, c * 512 + 512)
                gps = psM.tile([1, 512], F32, tag="psM", name=f"gps{c}_{h}")
                nc.tensor.matmul(
                    gps,
                    wg_sb[:, h : h + 1],
                    xg[:, gsl],
                    start=True,
                    stop=True,
                )
                gtmp = frp.tile([1, 512], F32, tag="dinv", name=f"gt{c}_{h}")
                nc.scalar.activation(gtmp, gps, AF.Exp, scale=-1.0)
                nc.vector.tensor_scalar_add(gtmp, gtmp, 1.0)
                nc.vector.reciprocal_approx_fast(gtmp, gtmp)
                nc.vector.tensor_copy(grows[h][:, gsl], gtmp)

        def vnew_part(b, i0, i1):
            boff = b * L
            if b not in vnews:
                vnews[b] = cachep.tile(
                    [128, NSC, HD], BF16, tag=f"vnew{b}", name=f"vn{b}"
                )
            vnew = vnews[b]
            for i in range(i0, i1):
                tp = psM.tile([128, 128], BF16, tag="psM", name=f"tp{b}_{i}")
                nc.tensor.transpose(
                    tp, qkvt[:, 3, boff + i * 128 : boff + i * 128 + 128], ident
                )
                nc.vector.tensor_copy(vnew[:, i, :], tp)

        # ---- phase 2: attention ---------------------------------------
        def attn_compute(b, h, lc2, drain_early=False):
            """Score/exp/den/pv stream plus the den->gate/den row chain.
            The PE-dependent epilogue is emitted separately (one iteration
            later) so the PE never waits on the off-PE chain."""
            it = (b * QH + h) * 2 + lc2
            boff = b * L
            off = boff + lc2 * 512
            ck_sb, cv_sb = cache_tiles[b]
            vnew = vnews[b]
            qsl = qkvt[:, h, off : off + 512]
            den = psD.tile([1, 512], F32, tag="psD", name=f"den{it}")
            ot = psO.tile([128, 512], F32, tag="psO", name=f"ot{it}")

            def den_pv(p, exs):
                # den first (1-col stationary, near-free LDW), then pv
                for hf in range(2):
                    sc = 2 * p + hf
                    nc.tensor.matmul(
                        den, ones_col, exs[:, hf, :],
                        start=(sc == 0), stop=(sc == NS - 1),
                    )
                for hf in range(2):
                    sc = 2 * p + hf
                    vx = cv_sb[:, sc, :] if sc < NSC else vnew[:, sc - NSC, :]
                    nc.tensor.matmul(
                        ot, vx, exs[:, hf, :],
                        start=(sc == 0), stop=(sc == NS - 1),
                    )

            pend = []
            for p in range(NS // 2):
                ps2 = psS.tile(
                    [128, 2, 512], F32, tag="psS", name=f"ps{it}_{p}"
                )
                exs = work.tile(
                    [128, 2, 512], BF16, tag="ex", name=f"ex{it}_{p}", bufs=5
                )
                for hf in range(2):
                    sc = 2 * p + hf
                    if sc < NSC:
                        kT = ck_sb[:, sc * 128 : sc * 128 + 128]
                    else:
                        j = boff + (sc - NSC) * 128
                        kT = qkvt[:, 2, j : j + 128]
                    nc.tensor.matmul(
                        ps2[:, hf, :], kT, qsl, start=True, stop=True
                    )
                # one exp over both chunks: N=1024 amortizes ACT overhead
                nc.scalar.activation(exs, ps2, AF.Exp)
                # pair p's den/pv run two score-pairs later so the PE
                # neither waits on the exp nor on the previous iteration's
                # denominator-bank read (depth-2 software pipeline)
                pend.append((p, exs))
                depth = 1 if drain_early else 3
                if len(pend) > depth:
                    den_pv(*pend.pop(0))
            for item in pend:
                den_pv(*item)
            # gate/den row: single-pass approx reciprocal of the den row,
            # times the matching gate row slice (DVE only, ~2us)
            dinv = frp.tile([1, 512], F32, tag="dinv", name=f"dinv{it}")
            nc.vector.reciprocal_approx_fast(dinv, den)
            frow = frp.tile([1, 512], F32R, tag="frow", name=f"frow{it}", bufs=2)
            nc.vector.tensor_mul(frow, dinv, grows[h][:, off : off + 512])
            return ot, frow

        def attn_epilogue(b, h, lc2, ot_frow):
            """PE-broadcast the gate/den row and apply it while evacuating
            ot -> otg (so phase 3 needs no per-head scaling)."""
            ot, frow = ot_frow
            it = (b * QH + h) * 2 + lc2
            bcf = psM.tile([128, 512], F32, tag="psM", name=f"bcf{it}")
            nc.tensor.matmul(bcf, ones_row, frow, start=True, stop=True)
            bcs = work.tile([128, 512], F32, tag="bcs", name=f"bcs{it}", bufs=2)
            nc.scalar.copy(bcs, bcf)
            nc.vector.tensor_mul(otg[:, b, h, lc2, :], ot, bcs)

        # ---- phase 3: partial out-projection --------------------------
        def phase3_block(b, lc2, li, act_ok=False):
            # double-width out-proj tiles: 2 column chunks per PSUM alloc
            # halves the pool churn, evacuations, and DMA count (2KB lines)
            row0 = b * L + lc2 * 512 + li * 128
            for mc2 in range(2):
                yp2 = psS.tile(
                    [128, 2, 512], F32, tag="psS",
                    name=f"yp{b}_{lc2}_{li}_{mc2}",
                )
                for half in range(2):
                    mc = 2 * mc2 + half
                    for h in range(QH):
                        nc.tensor.matmul(
                            yp2[:, half, :],
                            otg[:, b, h, lc2, li * 128 : li * 128 + 128],
                            wo_sb[:, h, mc * 512 : mc * 512 + 512],
                            start=(h == 0),
                            stop=(h == QH - 1),
                        )
                ysb2 = work.tile(
                    [128, 2, 512], BF16, tag="ysb", bufs=3,
                    name=f"ysb{b}_{lc2}_{li}_{mc2}",
                )
                # Copy is in every ACT table set, so ACT evacuation never
                # reloads tables; alternate ACT/DVE
                if (act_ok and mc2 == 0) or (not act_ok and mc2 == 1):
                    nc.scalar.copy(ysb2, yp2)
                else:
                    nc.vector.tensor_copy(ysb2, yp2)
                if act_ok:
                    eng = (nc.sync, nc.gpsimd, nc.scalar)[(2 * li + mc2) % 3]
                else:
                    eng = nc.sync if mc2 == 0 else nc.gpsimd
                eng.dma_start(
                    out=y[row0 : row0 + 128, mc2 * 1024 : mc2 * 1024 + 1024],
                    in_=ysb2,
                )

        # ---- emission order -------------------------------------------
        # q/k projections first (all rms sqrts complete before any exp --
        # keeps the ACT table-set switches to exactly two), then the v
        # projections: b0's during the proj->attention transition, b1's
        # woven into attention b0 alongside the rank-1 broadcasts
        for lc in range(4):
            proj_chunk(lc, (2, 0, 1))
        recip_rows(0)
        proj_chunk(4, (2, 0, 1))
        proj_chunk(5, (2, 0, 1))
        bc_pair(0, *BC_PAIRS[0])
        bc_pair(0, *BC_PAIRS[1])
        proj_chunk(6, (2, 0, 1))
        bc_pair(0, *BC_PAIRS[2])
        bc_pair(0, *BC_PAIRS[3])
        proj_chunk(7, (2, 0, 1))
        bc_pair(0, *BC_PAIRS[4])
        bc_pair(0, *BC_PAIRS[5])
        recip_rows(1)
        del xtiles[0]  # evicted by lc7 (7 bufs); re-DMA for the v pass
        for lc in (1, 2, 3, 0):
            proj_chunk(lc, (3,))
            vnew_part(0, 2 * lc, 2 * lc + 2)
        gates_part(0, 8)
        # attention b0; b1 v projections and half-1 broadcasts woven in
        ots = {}
        prev = None
        for i, (h, lc2) in enumerate([(0, 0), (1, 0), (0, 1), (1, 1)]):
            ots[(0, h, lc2)] = attn_compute(0, h, lc2)
            if prev is not None:
                attn_epilogue(*prev, ots[prev])
            for pair in BC_PAIRS[2 * i : 2 * i + 2]:
                bc_pair(1, *pair)
            proj_chunk(4 + i, (3,))
            vnew_part(1, 2 * i, 2 * i + 2)
            prev = (0, h, lc2)
        # attention b1 with phase-3 b0 blocks woven in: the out-proj's
        # pure-PE work fills the engine while b1's exps run on ACT
        p3_b0 = [(0, lc2, li) for lc2 in range(2) for li in range(4)]
        for i, (h, lc2) in enumerate([(0, 0), (1, 0), (0, 1), (1, 1)]):
            ots[(1, h, lc2)] = attn_compute(1, h, lc2, drain_early=(i == 3))
            attn_epilogue(*prev, ots[prev])
            prev = (1, h, lc2)
            for blk in p3_b0[i * 2 : i * 2 + 2]:
                phase3_block(*blk)
        # drain: b1 lc2=0 out-proj (both heads' otg ready) while the last
        # iteration's gate/den chain lands, then the lc2=1 tail
        for li in range(4):
            phase3_block(1, 0, li, act_ok=True)
        attn_epilogue(*prev, ots[prev])
        for li in range(4):
            phase3_block(1, 1, li, act_ok=True)

    nc.compile()
    return nc


def _get_nc():
    global _CACHED_NC
    if _CACHED_NC is None:
        _CACHED_NC = _build_core_program()
    return _CACHED_NC


def make_in_maps(x, w_q, w_k, w_v, w_out, w_gate, cache_k, cache_v):
    xt = x.reshape(BL, D).T.astype(NPBF16)
    consts_np = np.concatenate(
        [np.eye(128, dtype=np.float32), np.ones((128, 1), np.float32)], axis=1
    ).astype(NPBF16)
    onesr_np = np.ones((1, 128), np.float32)
    in_maps = []
    for c in range(NCORES):
        g = c // 2
        wq_c = w_q[c * JC : (c + 1) * JC]                      # [256, D]
        wk_c = w_k[g * HD : (g + 1) * HD]                      # [128, D]
        wv_c = w_v[g * HD : (g + 1) * HD]
        wqkv_c = (
            np.concatenate([wq_c, wk_c, wv_c], axis=0)
            .T.reshape(ND, 128, 4, HD)     # d = k*128 + p -> k, p, jq, hd
            .transpose(2, 1, 0, 3)          # [jq, p, k, hd]
            .astype(NPBF16)
        )
        wo_c = w_out[:, c * JC : (c + 1) * JC].T.astype(NPBF16)  # [256, D]
        wg_c = w_gate[c * QH : (c + 1) * QH].T.astype(NPBF16)    # [16, 2]
        ckt_c = cache_k[:, g].transpose(0, 2, 1).astype(NPBF16)  # [B,HD,CACHE]
        cv_c = cache_v[:, g].astype(NPBF16)                      # [B,CACHE,HD]
        in_maps.append(
            {
                "xt": xt,
                "wqkv": wqkv_c,
                "wo": wo_c,
                "wg": wg_c,
                "ckt": ckt_c,
                "cv": cv_c,
                "consts": consts_np,
                "onesr": onesr_np,
            }
        )
    return in_maps


def kernel(x, w_q, w_k, w_v, w_out, w_gate, cache_k, cache_v, _run_kwargs=None):
    in_maps = make_in_maps(x, w_q, w_k, w_v, w_out, w_gate, cache_k, cache_v)
    nc = _get_nc()
    res = run_bass_kernel_spmd(
        nc, in_maps, core_ids=list(range(NCORES)), **(_run_kwargs or {})
    )
    acc = np.zeros((BL, D), dtype=np.float32)
    for c in range(NCORES):
        acc += res.results[c]["y"].astype(np.float32)
    out = acc.reshape(B, L, D)
    if _run_kwargs:
        kernel.last_results = res
    return out
